# revision 100
# baseline (speedup 1.0000x reference)
"""DFNet (3-directional Mamba + 1x1 proj + MLP) Trainium2 Bass kernel, v2.

Per-core SPMD plan (8 cores):
  Each core owns 2 direction-slots (slot0: 4 state indices, slot1: 2); all
  48 (direction, n) pairs covered once across cores, both d_inner halves
  local to the owner.  The two channel LayerNorms fuse into one per-column
  affine (stats via PE reductions, column math on a folded [128,32] layout);
  conv1d folds into in_proj as 4 shifted matmul taps.  Selective scan runs
  as [128, 2048] tensor_tensor_scan strips chained over two L-halves.
  Gated contributions (out_proj+proj fused host-side) are summed across
  cores with one ReduceScatter; the MLP tail is token-parallel.
"""
import sys
for _p in ("/opt/trn_rl_repo", "/root/.axon_site/_ro/trn_rl_repo"):
    if _p not in sys.path:
        sys.path.insert(0, _p)

# ---- walrus workaround: split multi-sem-wait instructions ----
import concourse.tile as tile_mod
from concourse import mybir
from concourse.vector_clock import ScopedClock, VectorClock

_orig_add_instruction = tile_mod.TileContext._add_instruction
_split_counter = [0]


def _patched_add_instruction(self, inst):
    si = inst.sync_info
    if si is not None and inst.engine != mybir.EngineType.Unassigned:
        waits = list(si.on_wait or [])
        if len(waits) > 1:
            for w in waits[:-1]:
                _split_counter[0] += 1
                nop = mybir.InstNoOp(name=f"{inst.name}-ws{_split_counter[0]}")
                nop.engine = inst.engine
                nop.sync_info = mybir.SyncInfo(on_wait=[w], on_update=[])
                _orig_add_instruction(self, nop)
            inst.sync_info = mybir.SyncInfo(
                on_wait=[waits[-1]], on_update=list(si.on_update or [])
            )
    _orig_add_instruction(self, inst)


def _patched_drain_and_barrier(self, tick_clock, wait_clock):
    gc = tick_clock.global_clock
    n = len(gc)
    for i in range(n):
        t = gc[i]
        if t > 0:
            single = VectorClock([0] * n)
            single.require_at_least(i, t)
            d = self.nc.sync.drain()
            wait_clock.add_sem_waits(d.ins, ScopedClock({None: single}))
    self.nc.sync.drain()

    self.nc.all_engine_barrier()
    assert self.sems is not None
    popped = self.nc._tile_sem_poison_stack.pop()
    assert popped is self._sem_poison
    self.nc.clear_and_free_semaphores(list(self.sems.allocated().values()))
    self.nc.all_engine_barrier()


tile_mod.TileContext._add_instruction = _patched_add_instruction
tile_mod.TileContext._drain_and_barrier = _patched_drain_and_barrier

import numpy as np
from contextlib import ExitStack

import concourse.bass as bass
import concourse.tile as tile
from concourse.tile import add_dep_helper

FP32 = mybir.dt.float32
BF16 = mybir.dt.bfloat16
AF = mybir.ActivationFunctionType
ALU = mybir.AluOpType


class Dims:
    def __init__(self, C=128, E=16, n_cores=8):
        self.C = C
        self.E = E
        self.L = E ** 3            # 4096
        self.D_INNER = 2 * C       # 256
        self.NST = 16
        self.DT_RANK = (C + 15) // 16
        self.D_CONV = 4
        self.n_cores = n_cores
        self.LC = self.L // n_cores
        s0_dir = [0, 0, 0, 1, 1, 1, 2, 2]
        s0_n0 = [0, 4, 8, 0, 4, 8, 0, 4]
        s1_dir = [1, 1, 2, 2, 2, 2, 0, 0]
        s1_n0 = [12, 14, 8, 10, 12, 14, 12, 14]
        self.slots = []
        for c in range(n_cores):
            self.slots.append([
                (s0_dir[c], list(range(s0_n0[c], s0_n0[c] + 4))),
                (s1_dir[c], list(range(s1_n0[c], s1_n0[c] + 2))),
            ])
        cov = set()
        for c in range(n_cores):
            for g, ns in self.slots[c]:
                for n in ns:
                    assert (g, n) not in cov
                    cov.add((g, n))
        assert len(cov) == 3 * self.NST
        self.NSLOT = [4, 2]
        self.HB = self.L // 2      # half width 2048


def ref_forward_np(x, w):
    """Numpy replica of reference.py (float64 truth)."""
    C = x.shape[1]; E = x.shape[2]; L = E ** 3
    D_INNER = 2 * C; NST = 16; DT_RANK = (C + 15) // 16; D_CONV = 4
    x = x.astype(np.float64)
    g = {k: v.astype(np.float64) for k, v in w.items() if k != "x"}

    def ln_cf(t, wt, bt, eps=1e-6):
        u = t.mean(1, keepdims=True)
        s = ((t - u) ** 2).mean(1, keepdims=True)
        return wt[None, :, None, None, None] * ((t - u) / np.sqrt(s + eps)) \
            + bt[None, :, None, None, None]

    x5 = x.reshape(1, C, E, E, E)
    x1 = ln_cf(x5, g["ln_w"], g["ln_b"])
    xd = x1.reshape(1, C, L)
    xh = x1.transpose(0, 1, 3, 4, 2).reshape(1, C, L)
    xw = x1.transpose(0, 1, 4, 2, 3).reshape(1, C, L)
    seq = np.stack([xd, xh, xw], 0).reshape(3, C, L).swapaxes(1, 2)
    u_ = seq.mean(-1, keepdims=True)
    s_ = ((seq - u_) ** 2).mean(-1, keepdims=True)
    seq = (seq - u_) / np.sqrt(s_ + 1e-5) * g["mnorm_w"] + g["mnorm_b"]
    xz = seq @ g["in_proj_w"].T
    xr, z = xz[..., :D_INNER], xz[..., D_INNER:]
    xp = np.pad(xr, ((0, 0), (D_CONV - 1, 0), (0, 0)))
    xc = sum(g["conv_w"][:, k] * xp[:, k:k + L, :] for k in range(D_CONV)) + g["conv_b"]
    xc = xc * (1 / (1 + np.exp(-xc)))
    x_dbl = xc @ g["x_proj_w"].T
    dt = x_dbl[..., :DT_RANK]
    Bm = x_dbl[..., DT_RANK:DT_RANK + NST]
    Cm = x_dbl[..., DT_RANK + NST:]
    da = dt @ g["dt_proj_w"].T + g["dt_proj_b"]
    delta = np.log1p(np.exp(da))
    A = -np.exp(g["A_log"])
    N, Ln, d = xc.shape
    h = np.zeros((N, d, NST))
    ys = np.zeros((N, Ln, d))
    for t in range(Ln):
        dA = np.exp(delta[:, t, :, None] * A[None])
        dBu = delta[:, t, :, None] * Bm[:, t, None, :] * xc[:, t, :, None]
        h = dA * h + dBu
        ys[:, t] = np.einsum("bdn,bn->bd", h, Cm[:, t])
    y = ys + xc * g["D_param"]
    y = y * (z * (1 / (1 + np.exp(-z))))
    y = y @ g["out_proj_w"].T
    cat = y.swapaxes(1, 2).reshape(3, C, E, E, E)[None].transpose(1, 0, 2, 3, 4, 5)
    cat = cat.reshape(1, 3 * C, E, E, E)
    out1 = np.einsum("bkdhw,ok->bodhw", cat, g["proj_w"]) \
        + g["proj_b"][None, :, None, None, None]
    out_res = x5 + out1
    hh = ln_cf(out_res, g["ln_w"], g["ln_b"])
    hh = np.einsum("bcdhw,oc->bodhw", hh, g["fc1_w"]) + g["fc1_b"][None, :, None, None, None]
    from scipy.special import erf
    hh = hh * 0.5 * (1 + erf(hh / np.sqrt(2)))
    hh = np.einsum("bcdhw,oc->bodhw", hh, g["fc2_w"]) + g["fc2_b"][None, :, None, None, None]
    return (hh + out_res).astype(np.float32)


MB_COLS = 3782   # packed bf16 const columns
MF_COLS = 673    # packed fp32 const columns


def perms(E):
    A = np.arange(E ** 3).reshape(E, E, E)
    return [A.ravel(), A.transpose(1, 2, 0).ravel(), A.transpose(2, 0, 1).ravel()]


def make_small_inputs(dm, seed=0):
    rng = np.random.default_rng(seed)
    C, DI, RK, NST = dm.C, dm.D_INNER, dm.DT_RANK, dm.NST

    def w(shape, s=0.02):
        return (rng.standard_normal(shape) * s).astype(np.float32)

    return {
        "x": rng.standard_normal((1, C, dm.E, dm.E, dm.E)).astype(np.float32),
        "ln_w": np.ones(C, np.float32), "ln_b": np.zeros(C, np.float32),
        "mnorm_w": np.ones(C, np.float32), "mnorm_b": np.zeros(C, np.float32),
        "in_proj_w": w((2 * DI, C)),
        "conv_w": w((DI, 4), 0.2), "conv_b": np.zeros(DI, np.float32),
        "x_proj_w": w((RK + 2 * NST, DI)),
        "dt_proj_w": w((DI, RK), 0.1),
        "dt_proj_b": np.full(DI, float(np.log(np.expm1(0.01))), np.float32),
        "A_log": np.log(np.tile(np.arange(1, NST + 1, dtype=np.float32), (DI, 1))),
        "D_param": np.ones(DI, np.float32),
        "out_proj_w": w((C, DI)),
        "proj_w": w((C, 3 * C)),
        "proj_b": np.zeros(C, np.float32),
        "fc1_w": w((4 * C, C)), "fc1_b": np.zeros(4 * C, np.float32),
        "fc2_w": w((C, 4 * C)), "fc2_b": np.zeros(C, np.float32),
    }


def host_prep(dm, inputs):
    import ml_dtypes
    bf = ml_dtypes.bfloat16
    w = {k: np.asarray(v, np.float32) for k, v in inputs.items()}
    C, E, L, LC = dm.C, dm.E, dm.L, dm.LC
    DI, RK, NST = dm.D_INNER, dm.DT_RANK, dm.NST

    x2d = w["x"].reshape(C, L)
    Xg = np.stack([x2d[:, p] for p in perms(E)], 0)

    lnw, lnb = w["ln_w"], w["ln_b"]
    mw, mb = w["mnorm_w"], w["mnorm_b"]
    alpha = mw * lnw
    R = np.stack([mw * lnb, -mw * lnw, -mw, mb], 1)    # (C, 4)

    Wxr = w["in_proj_w"][0:DI, :]
    Wz = w["in_proj_w"][DI:2 * DI, :]
    convw = w["conv_w"]
    Ataps = []
    for k in range(4):
        Ak = (Wxr * convw[:, k:k + 1]).T * alpha[:, None]   # (128c, 256o)
        Ataps.append(np.ascontiguousarray(Ak.astype(bf)))
    WR = Wxr @ R
    Bcomb = np.zeros((16, DI), np.float32)
    kord = [3, 0, 1, 2]
    for kb in range(4):
        for j in range(4):
            Bcomb[kb * 4 + j, :] = convw[:, kord[kb]] * WR[:, j]
    Bcomb = Bcomb.astype(bf)
    Az = np.ascontiguousarray((Wz.T * alpha[:, None]).astype(bf))
    Bz = np.ascontiguousarray((Wz @ R).T.astype(bf))

    Wstat_x = np.stack([np.ones(C, np.float32), lnw, lnw * lnw, lnw * lnb], 1)
    Wstat_sq = np.stack([np.ones(C, np.float32), lnw * lnw], 1)

    statv = np.array([
        1.0 / C,                       # 0 invC
        -float(np.mean(lnw)),          # 1 -Lwbar
        float(np.mean(lnw * lnw)),     # 2 Lw2bar
        -float(np.mean(lnw * lnb)),    # 3 -Lwbbar
        float(np.mean(lnb)),           # 4 lnbbar
        float(np.mean(lnb * lnb)),     # 5 lnb2bar
    ], np.float32)
    statc = np.tile(statv[None, :], (C, 1))            # (128, 6)

    negA = np.exp(w["A_log"])                          # (256, 16) positive
    xproj = w["x_proj_w"]
    dtT = np.ascontiguousarray(w["dt_proj_w"].T.astype(bf))
    negdtb = np.ascontiguousarray((-w["dt_proj_b"])[:, None])

    out_proj = w["out_proj_w"]
    proj = w["proj_w"]

    fc1 = w["fc1_w"]
    fc1f = fc1 * lnw[None, :]
    fc1bf = w["fc1_b"] + fc1 @ lnb
    fc2 = w["fc2_w"]

    in_maps = []
    for c in range(dm.n_cores):
        xg = np.zeros((2, C, L + 3), np.float32)
        xpsel = np.zeros((2, 16, DI), np.float32)
        ascale = np.zeros((C, 12), np.float32)
        dcol = np.zeros((2, DI, 1), np.float32)
        wcomb = np.zeros((2, 2, C, C), np.float32)
        for s, (g, ns) in enumerate(dm.slots[c]):
            n_s = len(ns)
            xg[s, :, 3:] = Xg[g]
            for i in range(RK):
                xpsel[s, i, :] = xproj[i, :]
            for i, n in enumerate(ns):
                xpsel[s, 8 + i, :] = xproj[RK + n, :]
                xpsel[s, 12 + i, :] = xproj[RK + NST + n, :]
            for dh in range(2):
                for i, n in enumerate(ns):
                    col = dh * 4 + i if s == 0 else 8 + dh * 2 + i
                    ascale[:, col] = negA[dh * 128:(dh + 1) * 128, n]
                Wc = proj[:, g * C:(g + 1) * C] @ out_proj[:, dh * 128:(dh + 1) * 128]
                wcomb[s, dh] = Wc.T
                if s == 0 and 0 in ns:
                    dcol[s, dh * 128:(dh + 1) * 128, 0] = \
                        w["D_param"][dh * 128:(dh + 1) * 128]
        xpselT = xpsel.transpose(0, 2, 1)                  # (2, DI, 16)
        q = LC // 2
        xsl = np.concatenate([x2d[:, c * q:(c + 1) * q],
                              x2d[:, L // 2 + c * q:L // 2 + (c + 1) * q]], 1)

        # ---- pack all consts into one bf16 + one fp32 tensor ----
        mb = np.zeros((C, MB_COLS), np.float32)

        def putb(c0, arr):
            a = np.asarray(arr, np.float32)
            mb[:a.shape[0], c0:c0 + a.shape[1]] = a

        putb(0, Wstat_x)
        putb(4, Wstat_sq)
        for k in range(4):
            putb(6 + 256 * k, Ataps[k])
        putb(1030, Bcomb)
        putb(1286, Az)
        putb(1542, Bz)
        for s in range(2):
            for dh in range(2):
                putb(1798 + 16 * (2 * s + dh), xpselT[s, dh * 128:(dh + 1) * 128, :])
        putb(1862, dtT)
        putb(2118, np.ones((1, 128), np.float32))
        for s in range(2):
            for dh in range(2):
                putb(2246 + 128 * (2 * s + dh), wcomb[s, dh])
        putb(2758, fc1f.T)
        f2t = fc2.T
        for ot in range(4):
            putb(3270 + 128 * ot, f2t[ot * 128:(ot + 1) * 128, :])

        mf = np.zeros((C, MF_COLS), np.float32)

        def putf(c0, arr):
            a = np.asarray(arr, np.float32)
            mf[:a.shape[0], c0:c0 + a.shape[1]] = a

        putf(0, statc)
        putf(6, negdtb.reshape(2, 128).T)
        putf(8, w["conv_b"].reshape(2, 128).T)
        putf(10, np.full((C, 1), 1.0 / C, np.float32))
        putf(11, np.ones((1, 128), np.float32))
        putf(139, ascale)
        putf(151, dcol.reshape(4, 128).T)
        putf(155, w["proj_b"][:, None])
        putf(156, fc1bf.reshape(4, 128).T)
        putf(160, w["fc2_b"][:, None])
        putf(161, xsl)

        m = {
            "xg": xg.astype(bf),
            "megab": np.ascontiguousarray(mb.astype(bf)),
            "megaf": np.ascontiguousarray(mf),
            "zeros6": np.zeros((4, 8), ml_dtypes.bfloat16),
        }
        in_maps.append(m)
    return in_maps


def build_program(dm):
    C, E, L, LC = dm.C, dm.E, dm.L, dm.LC
    DI, RK, NST = dm.D_INNER, dm.DT_RANK, dm.NST
    NC = dm.n_cores
    HB = dm.HB                   # 2048
    RND = 1024
    NRH = HB // RND              # 2 rounds per half

    nc = bass.Bass()

    def inp(name, shape, dt=FP32):
        return nc.dram_tensor(name, list(shape), dt, kind="ExternalInput")

    xg = inp("xg", (2, C, L + 3), BF16)
    megab = inp("megab", (C, MB_COLS), BF16)
    megaf = inp("megaf", (C, MF_COLS))
    zeros6 = inp("zeros6", (4, 8), BF16)

    out_slice = nc.dram_tensor("out_slice", [C, LC], FP32, kind="ExternalOutput")

    prow_d = [nc.dram_tensor(f"prow{s}", [1, L], BF16) for s in range(2)]
    m4_d = [nc.dram_tensor(f"m4d{s}", [4, L + 6], BF16) for s in range(2)]
    bcr_d = [nc.dram_tensor(f"bcr{s}", [8, L], BF16) for s in range(2)]
    rs_in = [nc.dram_tensor(f"rs_in{h}", [NC, C, LC // 2], BF16)
             for h in range(2)]
    rs_out = [nc.dram_tensor(f"rs_out{h}", [C, LC // 2], BF16)
              for h in range(2)]

    with ExitStack() as ctx:
        tc = ctx.enter_context(tile.TileContext(nc))
        consts = ctx.enter_context(tc.tile_pool(name="consts", bufs=1))
        main = ctx.enter_context(tc.tile_pool(name="main", bufs=1))
        wps = ctx.enter_context(tc.tile_pool(name="wps", bufs=3, space="PSUM"))
        yps = ctx.enter_context(tc.tile_pool(name="yps", bufs=1, space="PSUM"))

        # ---------------- consts: packed loads (stats-critical cols first) ----
        megab_sb = consts.tile([C, MB_COLS], BF16, tag="megab", name="megab")
        megaf_sb = consts.tile([C, MF_COLS], FP32, tag="megaf", name="megaf")
        nc.scalar.dma_start(out=megab_sb[:, 0:6], in_=megab[:, 0:6])
        nc.scalar.dma_start(out=megaf_sb[:, 0:6], in_=megaf[:, 0:6])
        nc.scalar.dma_start(out=megab_sb[:, 6:MB_COLS], in_=megab[:, 6:MB_COLS])
        nc.scalar.dma_start(out=megaf_sb[:, 6:MF_COLS], in_=megaf[:, 6:MF_COLS])

        # xg loads first on the SP queue so stats can start immediately;
        # slot0 split in half so round-0 stat matmuls start sooner.
        xg_sb = {}
        HX = (L + 3) // 2 + 1
        t = main.tile([C, L + 3], BF16, tag="xg0", name="xg0")
        nc.sync.dma_start(out=t[:, 0:HX], in_=xg[0, :, 0:HX])
        nc.sync.dma_start(out=t[:, HX:], in_=xg[0, :, HX:])
        xg_sb[0] = t
        t = main.tile([C, L + 3], BF16, tag="xg1", name="xg1")
        nc.sync.dma_start(out=t, in_=xg[1, :, :])
        xg_sb[1] = t

        Wsx_sb = megab_sb[:, 0:4]
        Wsq_sb = megab_sb[:, 4:6]
        At_sb = [megab_sb[:, 6 + 256 * k:6 + 256 * (k + 1)] for k in range(4)]
        Bc_sb = megab_sb[0:16, 1030:1286]
        Az_sb = megab_sb[:, 1286:1542]
        Bz_sb = megab_sb[0:4, 1542:1798]
        xps_sb = {(s, dh): megab_sb[:, 1798 + 16 * (2 * s + dh):
                                    1798 + 16 * (2 * s + dh) + 16]
                  for s in range(2) for dh in range(2)}
        dtT_sb = megab_sb[0:RK, 1862:2118]
        ones_sb = megab_sb[0:1, 2118:2246]
        wc_sb = {(s, dh): megab_sb[:, 2246 + 128 * (2 * s + dh):
                                   2246 + 128 * (2 * s + dh) + 128]
                 for s in range(2) for dh in range(2)}
        fc1T_sb = megab_sb[:, 2758:3270]
        fc2T_sb = [megab_sb[:, 3270 + 128 * ot:3270 + 128 * (ot + 1)]
                   for ot in range(4)]

        statc_sb = megaf_sb[:, 0:6]
        negdtb_sb = [megaf_sb[:, 6 + dh:7 + dh] for dh in range(2)]
        convb_sb = [megaf_sb[:, 8 + dh:9 + dh] for dh in range(2)]
        ones32_sb = megaf_sb[:, 10:11]
        onesr32_sb = megaf_sb[0:1, 11:139]
        ascale_sb = megaf_sb[:, 139:151]
        dcol_sb = {(s, dh): megaf_sb[:, 151 + 2 * s + dh:152 + 2 * s + dh]
                   for s in range(2) for dh in range(2)}
        projb_sb = megaf_sb[:, 155:156]
        fc1b_sb = [megaf_sb[:, 156 + ot:157 + ot] for ot in range(4)]
        fc2b_sb = megaf_sb[:, 160:161]
        xres_sb = megaf_sb[:, 161:673]

        zds = []
        for s in range(2):
            z1 = nc.scalar.dma_start(out=m4_d[s][:, 0:3], in_=zeros6[:, 0:3])
            z2 = nc.scalar.dma_start(out=m4_d[s][:, L + 3:L + 6], in_=zeros6[:, 3:6])
            zds += [z1, z2]

        stat_deps = {}
        trh_cache = {}

        # ============ per-slot stats (full L, once) ============
        # Token-partitioned: chunk k (tokens 128k..128k+127) -> matmul with
        # lhsT = xg chunk, rhs = Wstat; psT[p, comp*32+k] = stat for token
        # tau = 128k + p.  No DRAM round trip, no PSUM-row copies.
        psT = yps.tile([128, RND], FP32, tag="yps", name="psT")

        def do_stats(s):
            co = s * 256         # column window inside psT for this slot
            for r in range(4):
                sqr = main.tile([C, RND], BF16, tag="sqr", name=f"sqr{s}{r}", bufs=2)
                c0 = 3 + r * RND
                nc.vector.tensor_tensor(sqr[:, :], xg_sb[s][:, c0:c0 + RND],
                                        xg_sb[s][:, c0:c0 + RND], ALU.mult)
                for kk in range(8):
                    k = r * 8 + kk
                    cc0 = 3 + 128 * k
                    out_x = bass.AP(tensor=psT.tensor,
                                    offset=psT.offset + co + k,
                                    ap=[list(psT.ap[0]), [32, 4]])
                    nc.tensor.matmul(out_x, xg_sb[s][:, cc0:cc0 + 128],
                                     Wsx_sb[:, :], start=True, stop=True)
                    out_q = bass.AP(tensor=psT.tensor,
                                    offset=psT.offset + co + 128 + k,
                                    ap=[list(psT.ap[0]), [32, 2]])
                    nc.tensor.matmul(out_q, sqr[:, kk * 128:(kk + 1) * 128],
                                     Wsq_sb[:, :], start=True, stop=True)
            statf = main.tile([128, 6 * 32], FP32, tag=f"stf{s}", name=f"statf{s}")
            nc.vector.tensor_copy(statf[:, :], psT[:, co:co + 192])
            return statf

        def stat_math(s, statf, half):
            # math + unfold for one L-half: cols k in [half*16, half*16+16)
            # of each 32-block (chunk k covers tokens 128k + p).
            off = half * 16

            def blk(t, i):
                return t[:, i * 32 + off:i * 32 + off + 16]
            S1, Sw, Sw2x, Swb, S2, S22 = (blk(statf, i) for i in range(6))
            sm = main.tile([128, 10 * 16], FP32, tag=f"sm{s}{half}",
                           name=f"sm{s}{half}")
            mu1, ex2, m1sq, rec1, r1, tt0, mu2, e1, q2, r2 = \
                (sm[:, i * 16:(i + 1) * 16] for i in range(10))
            V = nc.vector
            iC = statc_sb[:, 0:1]
            V.tensor_scalar(mu1, S1, iC, None, ALU.mult)
            V.tensor_scalar(ex2, S2, iC, None, ALU.mult)
            V.tensor_tensor(m1sq, mu1, mu1, ALU.mult)
            V.tensor_tensor(rec1, ex2, m1sq, ALU.subtract)
            V.tensor_scalar(rec1, rec1, 1e-6, None, ALU.add)
            V.reciprocal(rec1, rec1)
            nc.scalar.activation(r1, rec1, AF.Sqrt)
            # mu2 = r1*(Sw*invC - mu1*Lwbar) + lnbbar
            V.tensor_scalar(tt0, Sw, iC, None, ALU.mult)
            V.scalar_tensor_tensor(tt0, mu1, statc_sb[:, 1:2], tt0,
                                   ALU.mult, ALU.add)
            V.tensor_tensor(mu2, tt0, r1, ALU.mult)
            V.tensor_scalar(mu2, mu2, statc_sb[:, 4:5], None, ALU.add)
            # Ex12 = rec1*(S22*invC - 2 mu1 Sw2x invC + mu1^2 Lw2bar)
            #        + 2 r1 (Swb*invC - mu1*Lwbbar) + lnb2bar
            V.tensor_scalar(e1, Sw2x, iC, None, ALU.mult)
            V.tensor_tensor(e1, e1, mu1, ALU.mult)
            V.tensor_scalar(e1, e1, -2.0, None, ALU.mult)
            V.tensor_scalar(q2, S22, iC, None, ALU.mult)
            V.tensor_tensor(e1, e1, q2, ALU.add)
            V.tensor_scalar(q2, m1sq, statc_sb[:, 2:3], None, ALU.mult)
            V.tensor_tensor(e1, e1, q2, ALU.add)
            V.tensor_tensor(e1, e1, rec1, ALU.mult)
            V.tensor_scalar(q2, Swb, iC, None, ALU.mult)
            V.scalar_tensor_tensor(q2, mu1, statc_sb[:, 3:4], q2,
                                   ALU.mult, ALU.add)
            V.tensor_tensor(q2, q2, r1, ALU.mult)
            V.tensor_scalar(q2, q2, 2.0, None, ALU.mult)
            V.tensor_tensor(e1, e1, q2, ALU.add)
            V.tensor_scalar(e1, e1, statc_sb[:, 5:6], None, ALU.add)
            # var2 = Ex12 - mu2^2 ; r2 = 1/sqrt(var2+eps2)
            V.tensor_tensor(q2, mu2, mu2, ALU.mult)
            V.tensor_tensor(e1, e1, q2, ALU.subtract)
            V.tensor_scalar(e1, e1, 1e-5, None, ALU.add)
            V.reciprocal(e1, e1)
            nc.scalar.activation(r2, e1, AF.Sqrt)

            pm = main.tile([128, 128], BF16, tag=f"pm{s}{half}",
                           name=f"pm{s}{half}")
            Pf, M0, M1, M2, M3 = \
                (pm[:, i * 16:(i + 1) * 16] for i in range(5))
            V.tensor_tensor(Pf, r1, r2, ALU.mult)
            V.tensor_copy(M0, r2)
            V.tensor_tensor(M1, mu1, r1, ALU.mult)
            V.tensor_tensor(M1, M1, r2, ALU.mult)
            V.tensor_tensor(M2, mu2, r2, ALU.mult)
            V.memset(M3, 1.0)
            V.memset(pm[:, 80:128], 0.0)

            # XBAR transpose: trH[i, p] = pm[p, i]; i in [0,16) = P rows
            # (token 128k+p of this half), i in [16,80) = M_j rows.
            trH = main.tile([128, 128], BF16, tag=f"tr{s}{half}",
                            name=f"tr{s}{half}")
            tq = nc.sync
            tr_i = tq.dma_start_transpose(trH[:, :], pm[:, :])
            if half == 0 and s == 1:
                # keep slot1 unfold traffic ticked behind the slot0 h0 loads
                for (hh, ss), rds in pm_reads.items():
                    if hh == 0 and ss == 0:
                        for rd in rds:
                            add_dep_helper(tr_i.ins, rd.ins,
                                           reason="h1 unfold after h0 loads")
            trh_cache[(s, half)] = trH
            pw = nc.gpsimd.dma_start(
                out=prow_d[s][0, half * HB:half * HB + HB],
                in_=trH[0:16, :])
            # m4_d[s][j, 3 + half*HB + 128k + p] = trH[16 + j*16 + k, p]
            mw_ = nc.gpsimd.dma_start(
                out=bass.AP(tensor=m4_d[s], offset=3 + half * HB,
                            ap=[[L + 6, 4], [128, 16], [1, 128]]),
                in_=trH[16:80, :])
            stat_deps[(s, half)] = (pw, mw_)

        # ---- P_bc / M4 loads (hoistable so h0 reads beat slot1 traffic) ----
        pm_cache = {}
        pm_reads = {}

        def load_pm(h, s):
            if (h, s) in pm_cache:
                return pm_cache[(h, s)]
            o = h * HB
            pw0, mw0 = stat_deps[(s, 0)]
            if h == 1:
                pw1, mw1 = stat_deps[(s, 1)]
            pbc = main.tile([C, HB + 3], BF16, tag="pbc", name=f"pbc{h}{s}")
            if h == 0:
                nc.vector.memset(pbc[:, 0:3], 0.0)
                pr = nc.sync.dma_start(
                    out=pbc[:, 3:],
                    in_=bass.AP(tensor=prow_d[s], offset=0,
                                ap=[[0, 128], [1, HB]]))
                add_dep_helper(pr.ins, pw0.ins, reason="pbc after prow")
            else:
                pr = nc.sync.dma_start(
                    out=pbc[:, :],
                    in_=bass.AP(tensor=prow_d[s], offset=o - 3,
                                ap=[[0, 128], [1, HB + 3]]))
                add_dep_helper(pr.ins, pw0.ins, reason="pbc after prow h0")
                add_dep_helper(pr.ins, pw1.ins, reason="pbc after prow h1")
            m4 = main.tile([16, HB + 3], BF16, tag=f"m4{s}", name=f"m4{h}{s}")
            # rows 0:4 = unshifted (k=3); rows 4:16 = taps k=0,1,2
            if h == 0:
                # main reads stay within this half; the 3-col right tails
                # (first h1 tokens) load separately so they don't gate h0.
                mra = nc.sync.dma_start(
                    out=m4[0:4, 0:HB],
                    in_=bass.AP(tensor=m4_d[s], offset=o + 3,
                                ap=[[L + 6, 4], [1, HB]]))
                mrb = nc.sync.dma_start(
                    out=m4[4:16, 0:HB],
                    in_=bass.AP(tensor=m4_d[s], offset=o,
                                ap=[[1, 3], [L + 6, 4], [1, HB]]))
                add_dep_helper(mra.ins, mw0.ins, reason="m4 after unfold")
                for z_ in zds:
                    add_dep_helper(mrb.ins, z_.ins, reason="m4 halo zero")
                add_dep_helper(mrb.ins, mw0.ins, reason="m4 after unfold")
            else:
                mra = nc.sync.dma_start(
                    out=m4[0:4, :],
                    in_=bass.AP(tensor=m4_d[s], offset=o + 3,
                                ap=[[L + 6, 4], [1, HB + 3]]))
                mrb = nc.sync.dma_start(
                    out=m4[4:16, :],
                    in_=bass.AP(tensor=m4_d[s], offset=o,
                                ap=[[1, 3], [L + 6, 4], [1, HB + 3]]))
                for mr_ in (mra, mrb):
                    add_dep_helper(mr_.ins, mw0.ins, reason="m4 after unfold")
                    add_dep_helper(mr_.ins, mw1.ins, reason="m4 after unfold")
                    for z_ in zds:
                        add_dep_helper(mr_.ins, z_.ins, reason="m4 halo zero")
            pm_cache[(h, s)] = (pbc, m4)
            pm_reads[(h, s)] = [pr, mra, mrb]
            return pbc, m4

        def load_pm_tails(s):
            # Right-tail cols (c = HB..HB+2) of the h0 m4 tile, read straight
            # from the transpose tiles in SBUF (no DRAM round trip).
            #   rows 0:4   : m4[j, HB+c'] = M_j(HB + c')        <- trH1 k=0
            #   rows 4:16  : m4[4+4*k2+j, HB+c'] = M_j(HB-3+k2+c')
            #                tokens 2045..2049 span trH0 (k=15) / trH1 (k=0)
            _, m4 = pm_cache[(0, s)]
            trH0 = trh_cache[(s, 0)]
            trH1 = trh_cache[(s, 1)]
            PP0 = trH0.ap[0][0]
            PP1 = trH1.ap[0][0]

            def tr_ap(trH, PP, part0, col0, w):
                return bass.AP(tensor=trH.tensor,
                               offset=trH.offset + part0 * PP + col0,
                               ap=[[16 * PP, 4], [1, w]])

            # rows 0:4 <- M_j(2048..2050): trH1 parts 16+16j, cols 0:3
            nc.sync.dma_start(out=m4[0:4, HB:HB + 3],
                              in_=tr_ap(trH1, PP1, 16, 0, 3))
            for k2 in range(3):
                r0 = 4 + 4 * k2
                n0 = 3 - k2     # cols from trH0 (tokens < HB)
                if n0 > 0:
                    nc.sync.dma_start(
                        out=m4[r0:r0 + 4, HB:HB + n0],
                        in_=tr_ap(trH0, PP0, 31, 125 + k2, n0))
                if k2 > 0:
                    nc.sync.dma_start(
                        out=m4[r0:r0 + 4, HB + n0:HB + 3],
                        in_=tr_ap(trH1, PP1, 16, 0, k2))

        sf0 = do_stats(0)
        stat_math(0, sf0, 0)
        load_pm(0, 0)
        sf1 = do_stats(1)
        stat_math(1, sf1, 0)
        load_pm(0, 1)
        stat_math(0, sf0, 1)
        stat_math(1, sf1, 1)
        load_pm_tails(0)
        load_pm_tails(1)

        # persistent across halves
        hcar = main.tile([128, 16], BF16, tag="hcar")
        nc.vector.memset(hcar[:, :], 0.0)
        rs_writes = []

        # ============ main pipeline: halves x slots ============
        for h in range(2):
            o = h * HB           # global token offset
            ym_t = {}
            for s in range(2):
                n_s = dm.NSLOT[s]
                g_ns = dm.slots[0][s][1]  # structural only (len == n_s)
                pbc, m4 = load_pm(h, s)

                xP = main.tile([C, HB + 3], BF16, tag=f"xP{s}", name=f"xP{h}{s}")
                nc.vector.tensor_tensor(xP[:, :], xg_sb[s][:, o:o + HB + 3],
                                        pbc[:, :], ALU.mult)

                # ---- in_proj + conv + silu ----
                xc_t = {}
                sz_t = {}
                for dh in range(2):
                    xc_t[dh] = main.tile([128, HB], BF16, tag=f"xc{s}{dh}",
                                         name=f"xc{h}{s}{dh}")
                    sz_t[dh] = main.tile([128, HB], BF16, tag=f"sz{s}{dh}",
                                         name=f"sz{h}{s}{dh}")
                xdbl_sb = main.tile([16, HB], BF16, tag=f"xdbl{s}", name=f"xdbl{h}{s}")
                l_t = {}
                du_t = {}
                for dh in range(2):
                    l_t[dh] = main.tile([128, HB], BF16, tag=f"l{dh}",
                                        name=f"l{h}{s}{dh}")
                    du_t[dh] = main.tile([128, HB], BF16, tag=f"du{dh}",
                                         name=f"du{h}{s}{dh}")

                def ip_chunk(r, dh):
                    ps = wps.tile([128, RND], FP32, tag="work",
                                  name=f"ip{h}{s}{dh}{r}")
                    for ch in range(2):
                        o0 = r * RND + ch * 512
                        pslice = ps[:, ch * 512:(ch + 1) * 512]
                        for k in range(4):
                            nc.tensor.matmul(
                                pslice, At_sb[k][:, dh * 128:(dh + 1) * 128],
                                xP[:, o0 + k:o0 + k + 512],
                                start=(k == 0), stop=False)
                        nc.tensor.matmul(
                            pslice, Bc_sb[:, dh * 128:(dh + 1) * 128],
                            m4[:, 3 + o0:3 + o0 + 512],
                            start=False, stop=True)
                    nc.scalar.activation(
                        xc_t[dh][:, r * RND:(r + 1) * RND], ps[:, :],
                        AF.Silu, bias=convb_sb[dh][:, 0:1])

                def xd_chunk(r):
                    ps = wps.tile([128, RND], FP32, tag="work", name=f"xd{h}{s}{r}")
                    for ch in range(2):
                        o0 = r * RND + ch * 512
                        pslice = ps[0:16, ch * 512:(ch + 1) * 512]
                        for dh in range(2):
                            nc.tensor.matmul(pslice,
                                             xps_sb[(s, dh)][:, :],
                                             xc_t[dh][:, o0:o0 + 512],
                                             start=(dh == 0), stop=(dh == 1))
                    nc.scalar.copy(xdbl_sb[:, r * RND:(r + 1) * RND],
                                   ps[0:16, :])

                def da_chunk(r, dh):
                    ps = wps.tile([128, RND], FP32, tag="work",
                                  name=f"da{h}{s}{dh}{r}")
                    for ch in range(2):
                        o0 = r * RND + ch * 512
                        nc.tensor.matmul(ps[:, ch * 512:(ch + 1) * 512],
                                         dtT_sb[:, dh * 128:(dh + 1) * 128],
                                         xdbl_sb[0:8, o0:o0 + 512],
                                         start=True, stop=True)
                    nc.scalar.activation(
                        l_t[dh][:, r * RND:(r + 1) * RND], ps[:, :],
                        AF.Sigmoid, scale=-1.0,
                        bias=negdtb_sb[dh][:, 0:1])

                bws = []

                def bcr_write():
                    bw_ = nc.sync.dma_start(out=bcr_d[s][:, o:o + HB],
                                            in_=xdbl_sb[8:16, :])
                    bws.append(bw_)

                def ln_du(dh):
                    nc.scalar.activation(l_t[dh][:, :], l_t[dh][:, :], AF.Ln)
                    nc.vector.tensor_tensor(du_t[dh][:, :], l_t[dh][:, :],
                                            xc_t[dh][:, :], ALU.mult)

                if False:
                    # latency-optimized: full chunk pipeline per r
                    for r in range(NRH):
                        for dh in range(2):
                            ip_chunk(r, dh)
                        xd_chunk(r)
                        if r == NRH - 1:
                            bcr_write()
                        for dh in range(2):
                            da_chunk(r, dh)
                    for dh in range(2):
                        ln_du(dh)
                else:
                    # throughput-optimized phase order
                    for r in range(NRH):
                        for dh in range(2):
                            ip_chunk(r, dh)
                    for r in range(NRH):
                        xd_chunk(r)
                    bcr_write()
                    for dh in range(2):
                        for r in range(NRH):
                            da_chunk(r, dh)
                        ln_du(dh)

                # ---- strips (n outer, dh inner) ----
                y_sb = {}
                for dh in range(2):
                    y_sb[dh] = main.tile([128, HB], BF16, tag=f"ysb{dh}",
                                         name=f"ysb{h}{s}{dh}")
                    # init with D-term: y = xc*D  (we accumulate +hC since
                    # strip tensors carry a global minus sign: du=-delta*xc)
                    nc.vector.tensor_scalar(
                        y_sb[dh][:, :], xc_t[dh][:, :],
                        dcol_sb[(s, dh)][:, 0:1], None, ALU.mult)
                hC_acc = {0: [], 1: []}
                for ni in range(n_s):
                    bt = main.tile([128, HB], BF16, tag="bbcn",
                                   name=f"bbc{h}{s}{ni}", bufs=2)
                    b1 = nc.sync.dma_start(
                        out=bt,
                        in_=bass.AP(tensor=bcr_d[s], offset=ni * L + o,
                                    ap=[[0, 128], [1, HB]]))
                    for bw_ in bws:
                        add_dep_helper(b1.ins, bw_.ins, reason="bbc after bcr")
                    ct = main.tile([128, HB], BF16, tag="cbcn",
                                   name=f"cbc{h}{s}{ni}", bufs=2)
                    b2 = nc.sync.dma_start(
                        out=ct,
                        in_=bass.AP(tensor=bcr_d[s], offset=(4 + ni) * L + o,
                                    ap=[[0, 128], [1, HB]]))
                    for bw_ in bws:
                        add_dep_helper(b2.ins, bw_.ins, reason="cbc after bcr")
                    for dh in range(2):
                        col = dh * 4 + ni if s == 0 else 8 + dh * 2 + ni
                        dA = main.tile([128, HB], BF16, tag="dA",
                                       name=f"dA{h}{s}{dh}{ni}", bufs=3)
                        nc.scalar.activation(dA[:, :], l_t[dh][:, :], AF.Exp,
                                             scale=ascale_sb[:, col:col + 1])
                        dBu = main.tile([128, HB], BF16, tag="dBu",
                                        name=f"dBu{h}{s}{dh}{ni}", bufs=2)
                        nc.vector.tensor_tensor(dBu[:, :], du_t[dh][:, :],
                                                bt[:, :], ALU.mult)
                        hsc = main.tile([128, HB], BF16, tag="h",
                                        name=f"hs{h}{s}{dh}{ni}", bufs=2)
                        ci = (s * 8 + dh * 4 + ni)
                        init = 0.0 if h == 0 else hcar[:, ci:ci + 1]
                        nc.vector.tensor_tensor_scan(hsc[:, :], dA[:, :],
                                                     dBu[:, :], init,
                                                     ALU.mult, ALU.add)
                        if h == 0:
                            nc.vector.tensor_copy(hcar[:, ci:ci + 1],
                                                  hsc[:, HB - 1:HB])
                        hC = main.tile([128, HB], BF16, tag="hC",
                                       name=f"hC{h}{s}{dh}{ni}", bufs=3)
                        e2_ = nc.gpsimd if (ni == 0) else nc.vector
                        e2_.tensor_tensor(hC[:, :], hsc[:, :],
                                          ct[:, :], ALU.mult)
                        hC_acc[dh].append(hC)
                        if len(hC_acc[dh]) == 2:
                            a_, b_ = hC_acc[dh]
                            e4_ = nc.vector
                            e4_.tensor_tensor(a_[:, :], a_[:, :], b_[:, :],
                                              ALU.add)
                            nc.vector.tensor_tensor(y_sb[dh][:, :],
                                                    y_sb[dh][:, :], a_[:, :],
                                                    ALU.subtract)
                            hC_acc[dh] = []
                # ---- z projection (needed only at gating) ----
                for dh in range(2):
                    for r in range(NRH):
                        ps = wps.tile([128, RND], FP32, tag="work",
                                      name=f"z{h}{s}{dh}{r}")
                        for ch in range(2):
                            o0 = r * RND + ch * 512
                            pslice = ps[:, ch * 512:(ch + 1) * 512]
                            nc.tensor.matmul(pslice,
                                             Az_sb[:, dh * 128:(dh + 1) * 128],
                                             xP[:, 3 + o0:3 + o0 + 512],
                                             start=True, stop=False)
                            nc.tensor.matmul(pslice,
                                             Bz_sb[:, dh * 128:(dh + 1) * 128],
                                             m4[0:4, 3 + o0:3 + o0 + 512],
                                             start=False, stop=True)
                        nc.scalar.activation(
                            sz_t[dh][:, r * RND:(r + 1) * RND], ps[:, :], AF.Silu)

                for dh in range(2):
                    ym = main.tile([128, HB], BF16, tag=f"ym{s}{dh}",
                                   name=f"ym{h}{s}{dh}")
                    for r in range(NRH):
                        rc = slice(r * RND, (r + 1) * RND)
                        nc.vector.tensor_tensor(ym[:, rc], y_sb[dh][:, rc],
                                                sz_t[dh][:, rc], ALU.mult)
                    ym_t[(s, dh)] = ym

            # ---- contribution + ReduceScatter for this half ----
            h_writes = []
            for r in range(NRH):
                cp = yps.tile([128, RND], FP32, tag="yps", name=f"ct{h}{r}")
                units = [(s, dh) for s in range(2) for dh in range(2)]
                for ui, (s, dh) in enumerate(units):
                    for ch in range(2):
                        o0 = r * RND + ch * 512
                        nc.tensor.matmul(cp[:, ch * 512:(ch + 1) * 512],
                                         wc_sb[(s, dh)][:, :],
                                         ym_t[(s, dh)][:, o0:o0 + 512],
                                         start=(ui == 0), stop=(ui == 3))
                csb = main.tile([128, RND], BF16, tag="csb",
                                name=f"csb{h}{r}", bufs=2)
                nc.vector.tensor_copy(csb[:, :], cp[:, :])
                w_ = nc.sync.dma_start(
                    out=bass.AP(tensor=rs_in[h],
                                offset=r * 4 * (C * 256),
                                ap=[[256, 128], [C * 256, 4], [1, 256]]),
                    in_=csb[:, :])
                h_writes.append(w_)
            cc_h = nc.gpsimd.collective_compute(
                "ReduceScatter", ALU.add, replica_groups=[list(range(NC))],
                ins=[rs_in[h][:, :, :]], outs=[rs_out[h][:, :]])
            for w_ in h_writes:
                add_dep_helper(cc_h.ins, w_.ins, reason="rs after contrib")
            rs_writes.append(cc_h)

        # ---------- P3 (per RS half, first half hides under h1 compute) ----------
        Q = LC // 2
        for h in range(2):
            o1 = main.tile([C, Q], BF16, tag="o1", name=f"o1{h}")
            rd = nc.sync.dma_start(out=o1, in_=rs_out[h][:, :])
            add_dep_helper(rd.ins, rs_writes[h].ins, reason="read after rs")
            ores = main.tile([C, Q], FP32, tag="ores", name=f"ores{h}")
            nc.vector.scalar_tensor_tensor(
                ores[:, :], o1[:, :], projb_sb[:, 0:1],
                xres_sb[:, h * Q:(h + 1) * Q], ALU.add, ALU.add)
            sq3 = main.tile([C, Q], FP32, tag="obf", name=f"obf{h}")
            nc.scalar.activation(sq3[:, :], ores[:, :], AF.Square)
            stp = wps.tile([128, RND], FP32, tag="work", name=f"p3st{h}")
            nc.tensor.matmul(stp[0:1, 0:Q], ones32_sb[:, 0:1], ores[:, :],
                             start=True, stop=True)
            nc.tensor.matmul(stp[32:33, 0:Q], ones32_sb[:, 0:1], sq3[:, :],
                             start=True, stop=True)
            p3r = main.tile([1, 3 * Q], FP32, tag="p3rr", name=f"p3r{h}")
            mu_r = p3r[:, 0:Q]
            v_r = p3r[:, Q:2 * Q]
            musq_r = p3r[:, 2 * Q:3 * Q]
            rstd_r = p3r[:, 2 * Q:3 * Q]
            nc.vector.tensor_copy(mu_r, stp[0:1, 0:Q])
            nc.vector.tensor_tensor(musq_r, mu_r, mu_r, ALU.mult)
            nc.vector.tensor_tensor(v_r, stp[32:33, 0:Q], musq_r, ALU.subtract)
            nc.vector.tensor_scalar(v_r, v_r, 1e-6, None, ALU.add)
            nc.vector.reciprocal(v_r, v_r)
            nc.scalar.activation(rstd_r, v_r, AF.Sqrt)
            bc_ps = wps.tile([128, RND], FP32, tag="work", name=f"p3bc{h}")
            nc.tensor.matmul(bc_ps[:, 0:Q], onesr32_sb[0:1, :], mu_r,
                             start=True, stop=True)
            nc.tensor.matmul(bc_ps[:, 512:512 + Q], onesr32_sb[0:1, :], rstd_r,
                             start=True, stop=True)
            xh = main.tile([C, Q], FP32, tag="xh", name=f"xh{h}")
            nc.vector.tensor_tensor(xh[:, :], ores[:, :], bc_ps[:, 0:Q],
                                    ALU.subtract)
            xh2 = main.tile([C, Q], BF16, tag="xh2", name=f"xh2{h}")
            nc.vector.tensor_tensor(xh2[:, :], xh[:, :], bc_ps[:, 512:512 + Q],
                                    ALU.mult)
            gl = []
            for ot in range(4):
                f1p = wps.tile([128, RND], FP32, tag="work", name=f"f1{h}{ot}")
                nc.tensor.matmul(f1p[:, 0:Q],
                                 fc1T_sb[:, ot * 128:(ot + 1) * 128],
                                 xh2[:, :], start=True, stop=True)
                g_ = main.tile([128, Q], BF16, tag=f"g{ot}", name=f"g{ot}{h}")
                nc.scalar.activation(g_[:, :], f1p[:, 0:Q], AF.Gelu,
                                     bias=fc1b_sb[ot][:, 0:1])
                gl.append(g_)
            f2p = yps.tile([128, RND], FP32, tag="yps", name=f"f2{h}")
            for ot in range(4):
                nc.tensor.matmul(f2p[:, 0:Q], fc2T_sb[ot][:, :],
                                 gl[ot][:, :], start=(ot == 0), stop=(ot == 3))
            fin = main.tile([C, Q], FP32, tag="fin", name=f"fin{h}")
            nc.vector.scalar_tensor_tensor(
                fin[:, :], f2p[:, 0:Q], fc2b_sb[:, 0:1], ores[:, :],
                ALU.add, ALU.add)
            nc.sync.dma_start(out=out_slice[:, h * Q:(h + 1) * Q], in_=fin[:, :])

    return nc


def assemble_output(dm, results):
    C, E, L, LC = dm.C, dm.E, dm.L, dm.LC
    q = LC // 2
    out = np.zeros((C, L), np.float32)
    for c in range(dm.n_cores):
        r = results[c]["out_slice"]
        out[:, c * q:(c + 1) * q] = r[:, 0:q]
        out[:, L // 2 + c * q:L // 2 + (c + 1) * q] = r[:, q:LC]
    return out.reshape(1, C, E, E, E)


_CACHE = {}


def kernel(**inputs):
    dm = _CACHE.get("dm")
    if dm is None:
        dm = Dims(E=16)
        _CACHE["dm"] = dm
    nc = _CACHE.get("nc")
    if nc is None:
        nc = build_program(dm)
        _CACHE["nc"] = nc
    in_maps = host_prep(dm, inputs)
    from concourse.bass_utils import run_bass_kernel_spmd
    res = run_bass_kernel_spmd(nc, in_maps, list(range(dm.n_cores)))
    return assemble_output(dm, res.results)

